# revision 28
# baseline (speedup 1.0000x reference)
"""Trainium2 Bass kernel for causal multi-head attention.

Shapes (hardcoded): B=4, T=2048, D=1024, H=16, Dh=64, fp32 I/O.

Strategy (8 NeuronCores, tensor-parallel over heads):
  - Each core c owns heads (2c, 2c+1): computes Q^T/K^T/V projections for its
    128 head-dims over the whole [B*T, D] input (contracting D on the PE),
    then causal flash-style attention in "scores-transposed" orientation
    (S^T[k, q] blocks) so softmax needs no on-chip transposes:
      * exp on ScalarE, one merged instruction per key-block covering both
        heads ([128, 2, width] over a 2-bank PSUM group)
      * causal handling at 128-column granularity: S matmuls, exps and AV
        matmuls of the 4 diagonal blocks of each q-chunk are narrowed to the
        live query range; only the 128x128 boundary triangle gets a mask
      * denominator via a leading ones-column in the V stationary operand
        (row 0 of the AV psum = sum of exp weights)
      * division folded into the PSUM->SBUF cast against a PE-broadcast
        reciprocal
  - K^T is stored zero-padded per head ([128, 2, BT]) so every matmul in the
    kernel runs in the PE's 128x128 tile mode (no tiling-mode switches).
  - Projection / out-projection matmul groups are emitted as *filler* between
    attention blocks so the PE never idles (sustains the 2.4 GHz p-state).
  - An on-device AllToAll re-shards ctx^T from head-sharded to row-sharded,
    then each core computes out rows = ctx @ Wo + bo.

All matmul operands are fp16; accumulation is fp32 in PSUM.
"""

import sys

sys.path.insert(0, "/opt/trn_rl_repo")

import numpy as np

import concourse.bass as bass
import concourse.mybir as mybir
import concourse.tile as tile
from concourse import bacc
from concourse import bass_utils

N_CORES = 8
B, T, D, H, DH = 4, 2048, 1024, 16, 64
BT = B * T  # 8192
KS = D // 128  # 8 contraction subtiles
TC = 512  # t-chunk for projections
NTC = BT // TC  # 16
QC = 512  # query chunk in attention
NQC = T // QC  # 4 per batch
KB = 128  # key block
NKB = T // KB  # 16 per batch
ROWS = BT // N_CORES  # 1024 out rows per core
RB4 = ROWS // B  # 256 out rows per core per batch

F16 = mybir.dt.float16
F32 = mybir.dt.float32

_CACHE = {}


def _build():
    nc = bacc.Bacc("TRN2", target_bir_lowering=False, num_devices=N_CORES)

    # x pre-swizzled on host to [p, chunk, o, t] so each chunk DMA is fully
    # contiguous per partition (8KB lines instead of 1KB)
    x_d = nc.dram_tensor("x", [128, NTC, KS, TC], F16, kind="ExternalInput")
    # weights pre-swizzled on host to [p, o, h] / [p, r, n] (contiguous lines)
    wq_d = nc.dram_tensor("wq", [128, KS, 128], F16, kind="ExternalInput")
    wk_d = nc.dram_tensor("wk", [128, KS, 128], F16, kind="ExternalInput")
    wv_d = nc.dram_tensor("wv", [128, KS, 128], F16, kind="ExternalInput")
    wo_d = nc.dram_tensor("wo", [128, KS, D], F16, kind="ExternalInput")
    bo_d = nc.dram_tensor("bo", [D], F32, kind="ExternalInput")
    e2_d = nc.dram_tensor("e2", [128, 128], F16, kind="ExternalInput")
    cmask_d = nc.dram_tensor("cmask", [128, 2, 128], F16, kind="ExternalInput")
    out_d = nc.dram_tensor("out", [B, RB4, D], F32, kind="ExternalOutput")

    with tile.TileContext(nc) as tc:
        with (
            tc.tile_pool(name="persist", bufs=1) as persist,
            tc.tile_pool(name="xt", bufs=4) as xtp,
            tc.tile_pool(name="ep", bufs=8) as ep,
            tc.tile_pool(name="tail", bufs=2) as tailp,
            tc.tile_pool(name="ctx", bufs=3) as ctxp,
            tc.tile_pool(name="outp", bufs=3) as outp,
            tc.tile_pool(name="ps_s", bufs=2, space="PSUM") as ps_s,
            tc.tile_pool(name="ps_av", bufs=2, space="PSUM") as ps_av,
            tc.tile_pool(name="ps_misc", bufs=2, space="PSUM") as ps_misc,
            tc.tile_pool(name="dram", bufs=1, space="DRAM") as dram,
        ):
            # ---- persistent state ----
            wq_sb = persist.tile([128, KS, 128], F16)
            wk_sb = persist.tile([128, KS, 128], F16)
            wv_sb = persist.tile([128, KS, 128], F16)
            wo_sb = persist.tile([128, KS, D], F16)
            nc.sync.dma_start(wq_sb[:], wq_d[:])

            qt_sb = persist.tile([128, BT], F16)  # [2 heads x 64, global t]
            kt_sb = persist.tile([128, BT], F16)
            # V layout: [128 keys-in-block, B*NKB blocks, 2*(1+64)]
            #   per head h: cols 0:64 = V_h, col 64 = ones (denominator)
            v_sb = persist.tile([128, B * NKB, 2, DH + 1], F16)
            nc.vector.memset(v_sb[:, :, :, DH : DH + 1], 1.0)

            # bias broadcast [128, D] fp32 via PE ones-trick (DMAs + matmuls
            # emitted later so the critical wq/x DMAs issue first)
            ones_col = persist.tile([1, 128], F32)
            nc.vector.memset(ones_col[:], 1.0)
            bo_sb = persist.tile([1, D], F32)
            bias_sb = persist.tile([128, D], F32)

            # padded E2 selector (rows 0-63 <- r2[0], 64-127 <- r2[1]; rows
            # 2-127 of the moving operand are zero)
            e2_sb = persist.tile([128, 128], F16)
            r2hp = persist.tile([128, QC], F16)
            nc.vector.memset(r2hp[:], 0.0)

            # boundary triangle mask (both heads): cmask[p, h, j] = (j >= p)
            cmask_sb = persist.tile([128, 2, 128], F16)

            def emit_smalldmas_and_bias():
                nc.sync.dma_start(bo_sb[:], bo_d[None, :])
                nc.sync.dma_start(e2_sb[:], e2_d[:])
                nc.sync.dma_start(cmask_sb[:], cmask_d[:])
                for nch in range(2):
                    bps = ps_misc.tile([128, 512], F32, tag="misc")
                    nc.tensor.matmul(
                        bps[:], ones_col[:], bo_sb[:, nch * 512 : (nch + 1) * 512]
                    )
                    nc.vector.tensor_copy(
                        bias_sb[:, nch * 512 : (nch + 1) * 512], bps[:]
                    )

            # ---- projection emission (as filler items) ----
            def emit_xt_dma(tcn):
                xt = xtp.tile([128, KS, TC], F16, tag="xt", name="xt")
                nc.sync.dma_start(xt[:], x_d[:, tcn])
                return xt

            def emit_q_group(xt, tcn):
                t0 = tcn * TC
                pp = ps_misc.tile([128, TC], F32, tag="misc", name="qp")
                for ks in range(KS):
                    nc.tensor.matmul(
                        pp[:], wq_sb[:, ks, :], xt[:, ks, :],
                        start=(ks == 0), stop=(ks == KS - 1),
                    )
                nc.vector.tensor_copy(qt_sb[:, t0 : t0 + TC], pp[:])

            def emit_k_group(xt, tcn):
                t0 = tcn * TC
                pp = ps_misc.tile([128, TC], F32, tag="misc", name="kp")
                for ks in range(KS):
                    nc.tensor.matmul(
                        pp[:], wk_sb[:, ks, :], xt[:, ks, :],
                        start=(ks == 0), stop=(ks == KS - 1),
                    )
                nc.vector.tensor_copy(kt_sb[:, t0 : t0 + TC], pp[:])

            def emit_v_sub(xt, tcn, sub):
                vp = ps_misc.tile([128, 2, DH], F32, tag="misc", name="vp")
                for ks in range(KS):
                    nc.tensor.matmul(
                        vp[:],
                        xt[:, ks, sub * 128 : (sub + 1) * 128],
                        wv_sb[:, ks, :],
                        start=(ks == 0), stop=(ks == KS - 1),
                    )
                kbg = tcn * (TC // 128) + sub
                dst = v_sb[:, kbg, :, 0:DH]  # cols {0..63} u {65..128}
                nc.vector.tensor_copy(dst, vp[:])

            def proj_chunk_items(tcn):
                state = {}

                def first():
                    state["xt"] = emit_xt_dma(tcn)
                    emit_q_group(state["xt"], tcn)

                items = [first]
                items.append(lambda: emit_k_group(state["xt"], tcn))
                for sub in range(TC // 128):
                    items.append(
                        lambda s=sub: emit_v_sub(state["xt"], tcn, s)
                    )
                return items

            # ---- out-projection (as filler items) ----
            cc_ins = [dram.tile([N_CORES, 128, RB4], F16, name=f"cc_in{b}", tag=f"cc_in{b}") for b in range(B)]
            cc_outs = [dram.tile([N_CORES, 128, RB4], F16, name=f"cc_out{b}", tag=f"cc_out{b}") for b in range(B)]
            ao_sbs = []

            def emit_oproj_group(item):
                ob, oao, mb, nch = item
                t_in_ao = (mb * 128) % oao.shape[2]
                op = ps_misc.tile([128, 512], F32, tag="misc", name="op")
                for r in range(KS):
                    nc.tensor.matmul(
                        op[:],
                        oao[:, r, t_in_ao : t_in_ao + 128],
                        wo_sb[:, r, nch * 512 : (nch + 1) * 512],
                        start=(r == 0), stop=(r == KS - 1),
                    )
                osb = outp.tile([128, 512], F32, tag="osb", name="osb")
                nc.vector.tensor_tensor(
                    osb[:], op[:], bias_sb[:, nch * 512 : (nch + 1) * 512],
                    mybir.AluOpType.add,
                )
                nc.sync.dma_start(
                    out_d[ob, mb * 128 : (mb + 1) * 128,
                          nch * 512 : (nch + 1) * 512],
                    osb[:],
                )

            # ---- filler queue: keeps the PE fed between attention blocks.
            # Items carry a (b, qc) gate: not poppable before that position
            # (so an out-proj group never stalls the in-order PE behind its
            # AllToAll). cur_pos is updated by the attention loop. ----
            filler = []
            cur_pos = [0, 0]

            def pop_filler(n=1):
                popped = 0
                i = 0
                while popped < n and i < len(filler):
                    gate, fn = filler[i]
                    if gate <= (cur_pos[0], cur_pos[1]):
                        filler.pop(i)
                        fn()
                        popped += 1
                    else:
                        i += 1

            # batch 0 projections (+ chunks 4-5) emitted up front; first
            # x chunks' DMAs interleaved with the remaining weight DMAs so
            # the PE can start as soon as wq + xt0 land
            xt0 = emit_xt_dma(0)
            nc.sync.dma_start(wk_sb[:], wk_d[:])
            xt1 = emit_xt_dma(1)
            nc.sync.dma_start(wv_sb[:], wv_d[:])
            emit_smalldmas_and_bias()
            emit_q_group(xt0, 0)
            emit_k_group(xt0, 0)
            for sub in range(TC // 128):
                emit_v_sub(xt0, 0, sub)
            emit_q_group(xt1, 1)
            emit_k_group(xt1, 1)
            for sub in range(TC // 128):
                emit_v_sub(xt1, 1, sub)
            for tcn in range(2, 6):
                for it in proj_chunk_items(tcn):
                    it()
            nc.sync.dma_start(wo_sb[:], wo_d[:])
            # later chunks gated to when their batch's attention approaches, so
            # filler supply is spread across the whole attention phase
            for tcn in range(6, NTC):
                gate = (0, 0) if tcn < 8 else ((1, 0) if tcn < 12 else (2, 0))
                for it in proj_chunk_items(tcn):
                    filler.append((gate, it))

            # ---- attention ----
            pending_tail = []

            def emit_qc_tail_head(av0, av1):
                # Eagerly evacuate the av PSUM banks to SBUF (ACT for head0,
                # DVE for head1; both heads' dims packed into one [128, QC]
                # tile so the later SB-SB multiplies have matching base
                # partitions) so the next q-chunk's AV accumulation can claim
                # the banks immediately. Denominator rows go to partition-0
                # tiles for the DVE reciprocal chain.
                avc = tailp.tile([128, QC], F32, tag="avc")
                nc.vector.tensor_copy(avc[0:DH, :], av0[0:DH, :])
                nc.vector.tensor_copy(avc[DH : 2 * DH, :], av1[0:DH, :])
                d2a = tailp.tile([1, QC], F32, tag="d2a")
                d2b = tailp.tile([1, QC], F32, tag="d2b")
                nc.vector.tensor_copy(d2a[:], av0[DH : DH + 1, :])
                nc.vector.tensor_copy(d2b[:], av1[DH : DH + 1, :])
                r2a = tailp.tile([1, QC], F32, tag="r2a")
                r2b = tailp.tile([1, QC], F32, tag="r2b")
                nc.vector.reciprocal_approx_fast(r2a[:], d2a[:])
                nc.vector.reciprocal_approx_fast(r2b[:], d2b[:])
                nc.vector.tensor_copy(r2hp[0:1, :], r2a[:])
                nc.vector.tensor_copy(r2hp[64:65, :], r2b[:])
                return avc

            def emit_qc_tail(b, qc, avc):
                rb = ps_misc.tile([128, QC], F32, tag="misc", name="rb")
                nc.tensor.matmul(rb[:], e2_sb[:], r2hp[:])
                rb_sb = tailp.tile([128, QC], F16, tag="rbs")
                nc.vector.tensor_copy(rb_sb[:], rb[:])
                ctx2 = ctxp.tile([128, QC], F16, tag="ctx")
                nc.vector.tensor_tensor(
                    ctx2[0:64, :], avc[0:64, :], rb_sb[0:64, :],
                    mybir.AluOpType.mult,
                )
                nc.vector.tensor_tensor(
                    ctx2[64:128, :], avc[64:128, :], rb_sb[64:128, :],
                    mybir.AluOpType.mult,
                )
                s0 = qc * QC // RB4
                nc.sync.dma_start(
                    cc_ins[b][s0 : s0 + QC // RB4].rearrange("s p f -> p s f"),
                    ctx2[:].rearrange("p (s f) -> p s f", s=QC // RB4),
                )

            def emit_batch_a2a(b):
                nc.gpsimd.collective_compute(
                    "AllToAll",
                    mybir.AluOpType.bypass,
                    replica_groups=[list(range(N_CORES))],
                    ins=[cc_ins[b][:]],
                    outs=[cc_outs[b][:]],
                )
                ao_sb = persist.tile([128, KS, RB4], F16, name=f"ao{b}", tag=f"ao{b}")
                ao_sbs.append(ao_sb)
                nc.sync.dma_start(ao_sb[:], cc_outs[b].rearrange("r p t -> p r t"))
                # All earlier batches' out-proj groups are reserved as tail
                # filler: they keep the PE busy through the final batch's
                # AllToAll launch + transfer + rearrange (~30us)
                ogate = (B, 0)
                for mb in range(RB4 // 128):
                    for nch in range(2):
                        filler.append(
                            (ogate,
                             lambda a=ao_sb, m=mb, n=nch, bb=b: emit_oproj_group(
                                 (bb, a, m, n)
                             ))
                        )

            for b in range(B):
                for qc in range(NQC):
                    cur_pos[0], cur_pos[1] = b, qc
                    q0 = b * T + qc * QC
                    nkb = 4 * qc + 4
                    # boundary pop: the PE stalls here waiting for the prior
                    # q-chunk's last exps (scalar-engine lag); feed it filler
                    pop_filler(1)

                    def emit_se(kb):
                        # scores + merged exp (+ triangle mask) for block kb
                        k0 = b * T + kb * KB
                        diag_i = kb - 4 * qc  # >= 0 for diagonal blocks
                        lo = max(0, diag_i) * 128  # live query range start
                        sp = ps_s.tile([128, 2, QC], F32, tag="s", name="sp")
                        for h in (0, 1):
                            hs = slice(64 * h, 64 * (h + 1))
                            nc.tensor.matmul(
                                sp[:, h, lo:QC],
                                kt_sb[hs, k0 : k0 + KB],
                                qt_sb[hs, q0 + lo : q0 + QC],
                            )
                        e = ep.tile([128, 2, QC], F16, tag="e", name="e")
                        nc.scalar.activation(
                            e[:, :, lo:QC], sp[:, :, lo:QC],
                            mybir.ActivationFunctionType.Exp,
                            scale=0.125,
                        )
                        if diag_i >= 0:
                            nc.gpsimd.tensor_tensor(
                                e[:, :, lo : lo + 128],
                                e[:, :, lo : lo + 128],
                                cmask_sb[:],
                                mybir.AluOpType.mult,
                            )
                        return e, lo

                    # warm the S/exp pipeline, then flush the previous
                    # q-chunk's deferred tail (reads its av psum) BEFORE
                    # allocating this q-chunk's av tiles, then launch any
                    # collective whose inputs that tail produced
                    e_q = [emit_se(0)]
                    next_emit = 1
                    if nkb > 1:
                        e_q.append(emit_se(1))
                        next_emit = 2
                    while pending_tail:
                        pending_tail.pop(0)()
                    if qc == 0 and b > 0:
                        emit_batch_a2a(b - 1)

                    av0_full = ps_av.tile([128, QC], F32, tag="av", name="av0")
                    av1_full = ps_av.tile([128, QC], F32, tag="av", name="av1")
                    av0 = av0_full[: DH + 1]
                    av1 = av1_full[: DH + 1]

                    for kb in range(nkb):
                        kbg = b * NKB + kb
                        first, last = kb == 0, kb == nkb - 1
                        e_cur, lo = e_q.pop(0)
                        if kb > 0 and next_emit < nkb:
                            e_q.append(emit_se(next_emit))
                            next_emit += 1
                        nc.tensor.matmul(
                            av0[:, lo:QC], v_sb[:, kbg, 0, :], e_cur[:, 0, lo:QC],
                            start=first, stop=last,
                        )
                        if kb == 0 and next_emit < nkb:
                            # delay head1's first accumulation so the deferred
                            # division chain on DVE can release its av bank
                            e_q.append(emit_se(next_emit))
                            next_emit += 1
                        nc.tensor.matmul(
                            av1[:, lo:QC], v_sb[:, kbg, 1, :], e_cur[:, 1, lo:QC],
                            start=first, stop=last,
                        )
                        if kb % 2 == 1:
                            pop_filler(1)
                    avc = emit_qc_tail_head(av0, av1)
                    pending_tail.append(
                        lambda b_=b, qc_=qc, a_=avc: emit_qc_tail(b_, qc_, a_)
                    )

            while pending_tail:
                pending_tail.pop(0)()
            emit_batch_a2a(B - 1)
            cur_pos[0], cur_pos[1] = B, 0
            while filler:
                pop_filler(1)

    nc.compile()
    return nc


def _get_nc():
    if "nc" not in _CACHE:
        _CACHE["nc"] = _build()
    return _CACHE["nc"]


def _swizzle_w(w):
    # [D, N] -> [p, o, n] with d = o*128 + p, contiguous per partition
    w = np.asarray(w, dtype=np.float32).astype(np.float16)
    return np.ascontiguousarray(w.reshape(KS, 128, w.shape[1]).transpose(1, 0, 2))


def prepare_in_maps(x, Wq, Wk, Wv, Wo, bo):
    xT = np.asarray(x, dtype=np.float32).reshape(BT, D).T  # [D, BT]
    # [p, chunk, o, t] with d = o*128 + p
    x16 = np.ascontiguousarray(
        xT.reshape(KS, 128, NTC, TC).transpose(1, 2, 0, 3)
    ).astype(np.float16)
    wo16 = _swizzle_w(Wo)
    bo32 = np.ascontiguousarray(np.asarray(bo, dtype=np.float32))
    e2 = np.zeros((128, 128), dtype=np.float16)
    e2[0, 0:64] = 1.0
    e2[64, 64:128] = 1.0
    p = np.arange(128)[:, None]
    j = np.arange(128)[None, :]
    cmask = np.broadcast_to((j >= p).astype(np.float16)[:, None, :], (128, 2, 128))
    cmask = np.ascontiguousarray(cmask)
    in_maps = []
    for c in range(N_CORES):
        cs = slice(128 * c, 128 * (c + 1))
        in_maps.append(
            {
                "x": x16,
                "wq": _swizzle_w(np.asarray(Wq, np.float32)[:, cs]),
                "wk": _swizzle_w(np.asarray(Wk, np.float32)[:, cs]),
                "wv": _swizzle_w(np.asarray(Wv, np.float32)[:, cs]),
                "wo": wo16,
                "bo": bo32,
                "e2": e2,
                "cmask": cmask,
            }
        )
    return in_maps


def kernel(x, Wq, Wk, Wv, Wo, bo, _trace=False):
    nc = _get_nc()
    in_maps = prepare_in_maps(x, Wq, Wk, Wv, Wo, bo)
    res = bass_utils.run_bass_kernel_spmd(
        nc, in_maps, list(range(N_CORES)), trace=_trace
    )
    if _trace:
        _CACHE["last_results"] = res
    out = np.empty((B, T, D), dtype=np.float32)
    rb4 = ROWS // B
    for c in range(N_CORES):
        oc = res.results[c]["out"]  # [B, 256, D]
        for b in range(B):
            out[b, rb4 * c : rb4 * (c + 1), :] = oc[b]
    return out



# revision 37
# speedup vs baseline: 1.0098x; 1.0098x over previous
"""Trainium2 Bass kernel for causal multi-head attention.

Shapes (hardcoded): B=4, T=2048, D=1024, H=16, Dh=64, fp32 I/O.

Strategy (8 NeuronCores, tensor-parallel over heads):
  - Each core c owns heads (2c, 2c+1): computes Q^T/K^T/V projections for its
    128 head-dims over the whole [B*T, D] input (contracting D on the PE),
    then causal flash-style attention in "scores-transposed" orientation
    (S^T[k, q] blocks) so softmax needs no on-chip transposes:
      * exp on ScalarE, one merged instruction per key-block covering both
        heads ([128, 2, width] over a 2-bank PSUM group)
      * causal handling at 128-column granularity: S matmuls, exps and AV
        matmuls of the 4 diagonal blocks of each q-chunk are narrowed to the
        live query range; only the 128x128 boundary triangle gets a mask
      * denominator via a leading ones-column in the V stationary operand
        (row 0 of the AV psum = sum of exp weights)
      * division folded into the PSUM->SBUF cast against a PE-broadcast
        reciprocal
  - K^T is stored zero-padded per head ([128, 2, BT]) so every matmul in the
    kernel runs in the PE's 128x128 tile mode (no tiling-mode switches).
  - Projection / out-projection matmul groups are emitted as *filler* between
    attention blocks so the PE never idles (sustains the 2.4 GHz p-state).
  - An on-device AllToAll re-shards ctx^T from head-sharded to row-sharded,
    then each core computes out rows = ctx @ Wo + bo.

All matmul operands are fp16; accumulation is fp32 in PSUM.
"""

import sys

sys.path.insert(0, "/opt/trn_rl_repo")

import numpy as np

import concourse.bass as bass
import concourse.mybir as mybir
import concourse.tile as tile
from concourse import bacc
from concourse import bass_utils

N_CORES = 8
B, T, D, H, DH = 4, 2048, 1024, 16, 64
BT = B * T  # 8192
KS = D // 128  # 8 contraction subtiles
TC = 512  # t-chunk for projections
NTC = BT // TC  # 16
QC = 512  # query chunk in attention
NQC = T // QC  # 4 per batch
KB = 128  # key block
NKB = T // KB  # 16 per batch
ROWS = BT // N_CORES  # 1024 out rows per core
RB4 = ROWS // B  # 256 out rows per core per batch

F16 = mybir.dt.float16
F32 = mybir.dt.float32

_CACHE = {}


def _build():
    nc = bacc.Bacc("TRN2", target_bir_lowering=False, num_devices=N_CORES)

    # x pre-swizzled on host to [p, chunk, o, t] so each chunk DMA is fully
    # contiguous per partition (8KB lines instead of 1KB)
    x_d = nc.dram_tensor("x", [128, NTC, KS, TC], F16, kind="ExternalInput")
    # weights pre-swizzled on host to [p, o, h] / [p, r, n] (contiguous lines)
    wq_d = nc.dram_tensor("wq", [128, KS, 128], F16, kind="ExternalInput")
    wk_d = nc.dram_tensor("wk", [128, KS, 128], F16, kind="ExternalInput")
    wv_d = nc.dram_tensor("wv", [128, KS, 128], F16, kind="ExternalInput")
    wo_d = nc.dram_tensor("wo", [128, KS, D], F16, kind="ExternalInput")
    bo_d = nc.dram_tensor("bo", [D], F32, kind="ExternalInput")
    e2_d = nc.dram_tensor("e2", [128, 128], F16, kind="ExternalInput")
    cmask_d = nc.dram_tensor("cmask", [128, 2, 128], F16, kind="ExternalInput")
    out_d = nc.dram_tensor("out", [B, RB4, D], F32, kind="ExternalOutput")

    with tile.TileContext(nc) as tc:
        with (
            tc.tile_pool(name="persist", bufs=1) as persist,
            tc.tile_pool(name="xt", bufs=4) as xtp,
            tc.tile_pool(name="ep", bufs=8) as ep,
            tc.tile_pool(name="tail", bufs=2) as tailp,
            tc.tile_pool(name="ctx", bufs=3) as ctxp,
            tc.tile_pool(name="outp", bufs=3) as outp,
            tc.tile_pool(name="ps_s", bufs=2, space="PSUM") as ps_s,
            tc.tile_pool(name="ps_av", bufs=2, space="PSUM") as ps_av,
            tc.tile_pool(name="ps_misc", bufs=2, space="PSUM") as ps_misc,
            tc.tile_pool(name="dram", bufs=1, space="DRAM") as dram,
        ):
            # ---- persistent state ----
            wq_sb = persist.tile([128, KS, 128], F16)
            wk_sb = persist.tile([128, KS, 128], F16)
            wv_sb = persist.tile([128, KS, 128], F16)
            wo_sb = persist.tile([128, KS, D], F16)
            nc.sync.dma_start(wq_sb[:], wq_d[:])

            qt_sb = persist.tile([128, BT], F16)  # [2 heads x 64, global t]
            kt_sb = persist.tile([128, BT], F16)
            # V layout: [128 keys-in-block, B*NKB blocks, 2*(1+64)]
            #   per head h: cols 0:64 = V_h, col 64 = ones (denominator)
            v_sb = persist.tile([128, B * NKB, 2, DH + 1], F16)
            nc.vector.memset(v_sb[:, :, :, DH : DH + 1], 1.0)

            # bias broadcast [128, D] fp32 via PE ones-trick (DMAs + matmuls
            # emitted later so the critical wq/x DMAs issue first)
            ones_col = persist.tile([1, 128], F32)
            nc.vector.memset(ones_col[:], 1.0)
            bo_sb = persist.tile([1, D], F32)
            bias_sb = persist.tile([128, D], F32)

            # padded E2 selector (rows 0-63 <- r2[0], 64-127 <- r2[1]; rows
            # 2-127 of the moving operand are zero)
            e2_sb = persist.tile([128, 128], F16)
            r2hp = persist.tile([128, QC], F16)
            nc.vector.memset(r2hp[:], 0.0)

            # boundary triangle mask (both heads): cmask[p, h, j] = (j >= p)
            cmask_sb = persist.tile([128, 2, 128], F16)

            def emit_smalldmas_and_bias():
                nc.sync.dma_start(bo_sb[:], bo_d[None, :])
                nc.sync.dma_start(e2_sb[:], e2_d[:])
                nc.sync.dma_start(cmask_sb[:], cmask_d[:])
                for nch in range(2):
                    bps = ps_misc.tile([128, 512], F32, tag="misc")
                    nc.tensor.matmul(
                        bps[:], ones_col[:], bo_sb[:, nch * 512 : (nch + 1) * 512]
                    )
                    nc.vector.tensor_copy(
                        bias_sb[:, nch * 512 : (nch + 1) * 512], bps[:]
                    )

            # ---- projection emission (as filler items) ----
            def emit_xt_dma(tcn):
                xt = xtp.tile([128, KS, TC], F16, tag="xt", name="xt")
                nc.sync.dma_start(xt[:], x_d[:, tcn])
                return xt

            def emit_q_group(xt, tcn):
                t0 = tcn * TC
                pp = ps_misc.tile([128, TC], F32, tag="misc", name="qp")
                for ks in range(KS):
                    nc.tensor.matmul(
                        pp[:], wq_sb[:, ks, :], xt[:, ks, :],
                        start=(ks == 0), stop=(ks == KS - 1),
                    )
                nc.scalar.copy(qt_sb[:, t0 : t0 + TC], pp[:])

            def emit_k_group(xt, tcn):
                t0 = tcn * TC
                pp = ps_misc.tile([128, TC], F32, tag="misc", name="kp")
                for ks in range(KS):
                    nc.tensor.matmul(
                        pp[:], wk_sb[:, ks, :], xt[:, ks, :],
                        start=(ks == 0), stop=(ks == KS - 1),
                    )
                nc.vector.tensor_copy(kt_sb[:, t0 : t0 + TC], pp[:])

            def emit_v_sub(xt, tcn, sub):
                vp = ps_misc.tile([128, 2, DH], F32, tag="misc", name="vp")
                for ks in range(KS):
                    nc.tensor.matmul(
                        vp[:],
                        xt[:, ks, sub * 128 : (sub + 1) * 128],
                        wv_sb[:, ks, :],
                        start=(ks == 0), stop=(ks == KS - 1),
                    )
                kbg = tcn * (TC // 128) + sub
                dst = v_sb[:, kbg, :, 0:DH]  # cols {0..63} u {65..128}
                nc.vector.tensor_copy(dst, vp[:])

            def proj_chunk_items(tcn):
                state = {}

                def first():
                    state["xt"] = emit_xt_dma(tcn)
                    emit_q_group(state["xt"], tcn)

                items = [first]
                items.append(lambda: emit_k_group(state["xt"], tcn))
                for sub in range(TC // 128):
                    items.append(
                        lambda s=sub: emit_v_sub(state["xt"], tcn, s)
                    )
                return items

            # ---- out-projection (as filler items) ----
            cc_ins = [dram.tile([N_CORES, 128, RB4], F16, name=f"cc_in{b}", tag=f"cc_in{b}") for b in range(B - 1)]
            cc_outs = [dram.tile([N_CORES, 128, RB4], F16, name=f"cc_out{b}", tag=f"cc_out{b}") for b in range(B - 1)]
            cc_ins_h = [dram.tile([N_CORES, 128, RB4 // 2], F16, name=f"cc_inh{i}", tag=f"cc_inh{i}") for i in range(2)]
            cc_outs_h = [dram.tile([N_CORES, 128, RB4 // 2], F16, name=f"cc_outh{i}", tag=f"cc_outh{i}") for i in range(2)]
            ao_sbs = []

            def emit_oproj_group(item):
                ob, oao, mb, nch = item
                t_in_ao = (mb * 128) % oao.shape[2]
                op = ps_misc.tile([128, 512], F32, tag="misc", name="op")
                for r in range(KS):
                    nc.tensor.matmul(
                        op[:],
                        oao[:, r, t_in_ao : t_in_ao + 128],
                        wo_sb[:, r, nch * 512 : (nch + 1) * 512],
                        start=(r == 0), stop=(r == KS - 1),
                    )
                osb = outp.tile([128, 512], F32, tag="osb", name="osb")
                nc.vector.tensor_tensor(
                    osb[:], op[:], bias_sb[:, nch * 512 : (nch + 1) * 512],
                    mybir.AluOpType.add,
                )
                nc.sync.dma_start(
                    out_d[ob, mb * 128 : (mb + 1) * 128,
                          nch * 512 : (nch + 1) * 512],
                    osb[:],
                )

            # ---- filler queue: keeps the PE fed between attention blocks.
            # Items carry a (b, qc) gate: not poppable before that position
            # (so an out-proj group never stalls the in-order PE behind its
            # AllToAll). cur_pos is updated by the attention loop. ----
            filler = []
            cur_pos = [0, 0]

            def pop_filler(n=1):
                popped = 0
                i = 0
                while popped < n and i < len(filler):
                    gate, fn = filler[i]
                    if gate <= (cur_pos[0], cur_pos[1]):
                        filler.pop(i)
                        fn()
                        popped += 1
                    else:
                        i += 1

            # batch 0 projections (+ chunks 4-5) emitted up front; first
            # x chunks' DMAs interleaved with the remaining weight DMAs so
            # the PE can start as soon as wq + xt0 land
            xt0 = emit_xt_dma(0)
            nc.sync.dma_start(wk_sb[:], wk_d[:])
            xt1 = emit_xt_dma(1)
            nc.sync.dma_start(wv_sb[:], wv_d[:])
            emit_smalldmas_and_bias()
            emit_q_group(xt0, 0)
            emit_k_group(xt0, 0)
            for sub in range(TC // 128):
                emit_v_sub(xt0, 0, sub)
            emit_q_group(xt1, 1)
            emit_k_group(xt1, 1)
            for sub in range(TC // 128):
                emit_v_sub(xt1, 1, sub)
            for tcn in range(2, 6):
                for it in proj_chunk_items(tcn):
                    it()
            nc.sync.dma_start(wo_sb[:], wo_d[:])
            # later chunks gated to when their batch's attention approaches, so
            # filler supply is spread across the whole attention phase
            for tcn in range(6, NTC):
                gate = (0, 0) if tcn < 8 else ((1, 0) if tcn < 12 else (2, 0))
                for it in proj_chunk_items(tcn):
                    filler.append((gate, it))

            def emit_half_a2a(half):
                nc.gpsimd.collective_compute(
                    "AllToAll",
                    mybir.AluOpType.bypass,
                    replica_groups=[list(range(N_CORES))],
                    ins=[cc_ins_h[half][:]],
                    outs=[cc_outs_h[half][:]],
                )
                RBH = RB4 // 2
                ao_sb = persist.tile([128, KS, RBH], F16, name=f"aoh{half}", tag=f"aoh{half}")
                ao_sbs.append(ao_sb)
                nc.sync.dma_start(ao_sb[:], cc_outs_h[half].rearrange("r p t -> p r t"))
                for nch in range(2):
                    filler.append(
                        ((B, 0),
                         lambda a=ao_sb, h=half, n=nch: emit_oproj_group((B - 1, a, h, n)))
                    )

            # ---- attention ----
            pending_tail = []

            def emit_qc_tail_head(av0, av1):
                # Eagerly evacuate the av PSUM banks to SBUF (ACT for head0,
                # DVE for head1; both heads' dims packed into one [128, QC]
                # tile so the later SB-SB multiplies have matching base
                # partitions) so the next q-chunk's AV accumulation can claim
                # the banks immediately. Denominator rows go to partition-0
                # tiles for the DVE reciprocal chain.
                avc = tailp.tile([128, QC], F32, tag="avc")
                nc.scalar.copy(avc[0:DH, :], av0[0:DH, :])
                nc.vector.tensor_copy(avc[DH : 2 * DH, :], av1[0:DH, :])
                d2a = tailp.tile([1, QC], F32, tag="d2a")
                d2b = tailp.tile([1, QC], F32, tag="d2b")
                nc.vector.tensor_copy(d2a[:], av0[DH : DH + 1, :])
                nc.vector.tensor_copy(d2b[:], av1[DH : DH + 1, :])
                r2a = tailp.tile([1, QC], F32, tag="r2a")
                r2b = tailp.tile([1, QC], F32, tag="r2b")
                nc.vector.reciprocal_approx_fast(r2a[:], d2a[:])
                nc.vector.reciprocal_approx_fast(r2b[:], d2b[:])
                nc.vector.tensor_copy(r2hp[0:1, :], r2a[:])
                nc.vector.tensor_copy(r2hp[64:65, :], r2b[:])
                return avc

            def emit_qc_tail(b, qc, avc):
                rb = ps_misc.tile([128, QC], F32, tag="misc", name="rb")
                nc.tensor.matmul(rb[:], e2_sb[:], r2hp[:])
                rb_sb = tailp.tile([128, QC], F16, tag="rbs")
                nc.vector.tensor_copy(rb_sb[:], rb[:])
                ctx2 = ctxp.tile([128, QC], F16, tag="ctx")
                nc.vector.tensor_tensor(
                    ctx2[0:64, :], avc[0:64, :], rb_sb[0:64, :],
                    mybir.AluOpType.mult,
                )
                nc.vector.tensor_tensor(
                    ctx2[64:128, :], avc[64:128, :], rb_sb[64:128, :],
                    mybir.AluOpType.mult,
                )
                if b < B - 1:
                    s0 = qc * QC // RB4
                    nc.sync.dma_start(
                        cc_ins[b][s0 : s0 + QC // RB4].rearrange("s p f -> p s f"),
                        ctx2[:].rearrange("p (s f) -> p s f", s=QC // RB4),
                    )
                else:
                    half, RBH = qc // 2, RB4 // 2
                    s0 = (qc % 2) * QC // RBH
                    nc.sync.dma_start(
                        cc_ins_h[half][s0 : s0 + QC // RBH].rearrange("s p f -> p s f"),
                        ctx2[:].rearrange("p (s f) -> p s f", s=QC // RBH),
                    )

            def emit_batch_a2a(b):
                nc.gpsimd.collective_compute(
                    "AllToAll",
                    mybir.AluOpType.bypass,
                    replica_groups=[list(range(N_CORES))],
                    ins=[cc_ins[b][:]],
                    outs=[cc_outs[b][:]],
                )
                ao_sb = persist.tile([128, KS, RB4], F16, name=f"ao{b}", tag=f"ao{b}")
                ao_sbs.append(ao_sb)
                nc.sync.dma_start(ao_sb[:], cc_outs[b].rearrange("r p t -> p r t"))
                # batch 1's and 2's groups are reserved as tail filler: they
                # keep the PE busy while the final half-batch AllToAlls fly
                ogate = (b + 1, 1) if b < 1 else (B, 0)
                for mb in range(RB4 // 128):
                    for nch in range(2):
                        filler.append(
                            (ogate,
                             lambda a=ao_sb, m=mb, n=nch, bb=b: emit_oproj_group(
                                 (bb, a, m, n)
                             ))
                        )

            for b in range(B):
                for qc in range(NQC):
                    cur_pos[0], cur_pos[1] = b, qc
                    q0 = b * T + qc * QC
                    nkb = 4 * qc + 4
                    # boundary pop: the PE stalls here waiting for the prior
                    # q-chunk's last exps (scalar-engine lag); feed it filler
                    pop_filler(1)

                    def emit_se(kb):
                        # scores + merged exp (+ triangle mask) for block kb
                        k0 = b * T + kb * KB
                        diag_i = kb - 4 * qc  # >= 0 for diagonal blocks
                        lo = max(0, diag_i) * 128  # live query range start
                        sp = ps_s.tile([128, 2, QC], F32, tag="s", name="sp")
                        for h in (0, 1):
                            hs = slice(64 * h, 64 * (h + 1))
                            nc.tensor.matmul(
                                sp[:, h, lo:QC],
                                kt_sb[hs, k0 : k0 + KB],
                                qt_sb[hs, q0 + lo : q0 + QC],
                            )
                        e = ep.tile([128, 2, QC], F16, tag="e", name="e")
                        nc.scalar.activation(
                            e[:, :, lo:QC], sp[:, :, lo:QC],
                            mybir.ActivationFunctionType.Exp,
                            scale=0.125,
                        )
                        if diag_i >= 0:
                            nc.gpsimd.tensor_tensor(
                                e[:, :, lo : lo + 128],
                                e[:, :, lo : lo + 128],
                                cmask_sb[:],
                                mybir.AluOpType.mult,
                            )
                        return e, lo

                    # warm the S/exp pipeline, then flush the previous
                    # q-chunk's deferred tail (reads its av psum) BEFORE
                    # allocating this q-chunk's av tiles, then launch any
                    # collective whose inputs that tail produced
                    e_q = [emit_se(0)]
                    next_emit = 1
                    if nkb > 1:
                        e_q.append(emit_se(1))
                        next_emit = 2
                    while pending_tail:
                        pending_tail.pop(0)()
                    if qc == 0 and b > 0:
                        emit_batch_a2a(b - 1)
                    if b == B - 1 and qc == 2:
                        emit_half_a2a(0)

                    av0_full = ps_av.tile([128, QC], F32, tag="av", name="av0")
                    av1_full = ps_av.tile([128, QC], F32, tag="av", name="av1")
                    av0 = av0_full[: DH + 1]
                    av1 = av1_full[: DH + 1]

                    for kb in range(nkb):
                        kbg = b * NKB + kb
                        first, last = kb == 0, kb == nkb - 1
                        e_cur, lo = e_q.pop(0)
                        if kb > 0 and next_emit < nkb:
                            e_q.append(emit_se(next_emit))
                            next_emit += 1
                        nc.tensor.matmul(
                            av0[:, lo:QC], v_sb[:, kbg, 0, :], e_cur[:, 0, lo:QC],
                            start=first, stop=last,
                        )
                        if kb == 0 and next_emit < nkb:
                            # delay head1's first accumulation so the deferred
                            # division chain on DVE can release its av bank
                            e_q.append(emit_se(next_emit))
                            next_emit += 1
                        nc.tensor.matmul(
                            av1[:, lo:QC], v_sb[:, kbg, 1, :], e_cur[:, 1, lo:QC],
                            start=first, stop=last,
                        )
                        if kb % 2 == 1:
                            pop_filler(1)
                    avc = emit_qc_tail_head(av0, av1)
                    pending_tail.append(
                        lambda b_=b, qc_=qc, a_=avc: emit_qc_tail(b_, qc_, a_)
                    )

            while pending_tail:
                pending_tail.pop(0)()
            emit_half_a2a(1)
            cur_pos[0], cur_pos[1] = B, 0
            while filler:
                pop_filler(1)

    nc.compile()
    return nc


def _get_nc():
    if "nc" not in _CACHE:
        _CACHE["nc"] = _build()
    return _CACHE["nc"]


def _swizzle_w(w):
    # [D, N] -> [p, o, n] with d = o*128 + p, contiguous per partition
    w = np.asarray(w, dtype=np.float32).astype(np.float16)
    return np.ascontiguousarray(w.reshape(KS, 128, w.shape[1]).transpose(1, 0, 2))


def prepare_in_maps(x, Wq, Wk, Wv, Wo, bo):
    xT = np.asarray(x, dtype=np.float32).reshape(BT, D).T  # [D, BT]
    # [p, chunk, o, t] with d = o*128 + p
    x16 = np.ascontiguousarray(
        xT.reshape(KS, 128, NTC, TC).transpose(1, 2, 0, 3)
    ).astype(np.float16)
    wo16 = _swizzle_w(Wo)
    bo32 = np.ascontiguousarray(np.asarray(bo, dtype=np.float32))
    e2 = np.zeros((128, 128), dtype=np.float16)
    e2[0, 0:64] = 1.0
    e2[64, 64:128] = 1.0
    p = np.arange(128)[:, None]
    j = np.arange(128)[None, :]
    cmask = np.broadcast_to((j >= p).astype(np.float16)[:, None, :], (128, 2, 128))
    cmask = np.ascontiguousarray(cmask)
    in_maps = []
    for c in range(N_CORES):
        cs = slice(128 * c, 128 * (c + 1))
        in_maps.append(
            {
                "x": x16,
                "wq": _swizzle_w(np.asarray(Wq, np.float32)[:, cs]),
                "wk": _swizzle_w(np.asarray(Wk, np.float32)[:, cs]),
                "wv": _swizzle_w(np.asarray(Wv, np.float32)[:, cs]),
                "wo": wo16,
                "bo": bo32,
                "e2": e2,
                "cmask": cmask,
            }
        )
    return in_maps


def kernel(x, Wq, Wk, Wv, Wo, bo, _trace=False):
    nc = _get_nc()
    in_maps = prepare_in_maps(x, Wq, Wk, Wv, Wo, bo)
    res = bass_utils.run_bass_kernel_spmd(
        nc, in_maps, list(range(N_CORES)), trace=_trace
    )
    if _trace:
        _CACHE["last_results"] = res
    out = np.empty((B, T, D), dtype=np.float32)
    rb4 = ROWS // B
    rbh = rb4 // 2
    for c in range(N_CORES):
        oc = res.results[c]["out"]  # [B, 256, D]
        for b in range(B - 1):
            out[b, rb4 * c : rb4 * (c + 1), :] = oc[b]
        # last batch was exchanged as two half-batch A2As with 128-row shards
        out[B - 1, rbh * c : rbh * (c + 1), :] = oc[B - 1, 0:rbh]
        out[B - 1, T // 2 + rbh * c : T // 2 + rbh * (c + 1), :] = oc[B - 1, rbh:]
    return out

